# revision 20
# baseline (speedup 1.0000x reference)
"""MoE routing kernel for Trainium2 (8 NeuronCores, Bass/Tile).

Problem: nn_MoE (B=16384, I=1024, H=4096, E=8, top-2, difficulty-conditioned
gate).  Strategy: data-parallel over tokens (2048 tokens/core), experts
replicated and streamed from HBM.  Per core, routing (gate matmul, softmax,
top-2, expert grouping) is computed on-device; tokens are gathered by expert
assignment via SWDGE dma_gather, matmul'd against each expert's weights in
fp32r (one pass per 128-column i-chunk, N=512 moving), scaled by their gate
weight (folded into the gathered activations), bias applied via a k=1 rank-1
matmul, and combined back into token order with dma_scatter_add.

Host side only does layout prep (weight transpose, constant folding of the
difficulty-embedding x gate-weight product) and slicing/concat of I/O.
"""

import numpy as np

import concourse.bass as bass
import concourse.bacc as bacc
import concourse.mybir as mybir
import concourse.tile as tile
from concourse.bass_utils import run_bass_kernel_spmd
from concourse.masks import make_identity

# Problem constants (hardcoded per harness contract)
B = 16384
I = 1024
H = 4096
E = 8
TOPK = 2
NCORES = 8
BC = B // NCORES          # tokens per core = 2048
NT = BC // 128            # gate tiles per core = 16
CAP = 640                 # per-expert slot capacity per core (5 tiles of 128)
KT = CAP // 128           # expert tiles = 5
HC = 512                  # h-chunk width (one PSUM bank)
NHC = H // HC             # 8 h-chunks
IC = I // 128             # 8 i-chunks
BCP = BC + 128            # +128 pad rows: pad slots gather/scatter row BC (dummy)

f32 = mybir.dt.float32
f32r = mybir.dt.float32r
i32 = mybir.dt.int32
u32 = mybir.dt.uint32
i16 = mybir.dt.int16
AL = mybir.AluOpType


def build_nc():
    nc = bacc.Bacc("TRN2", target_bir_lowering=False, debug=False)

    xs = nc.dram_tensor("xs", [BCP, I], f32, kind="ExternalInput")
    labf = nc.dram_tensor("labf", [BC, 1], f32, kind="ExternalInput")
    wT = nc.dram_tensor("wT", [E, I, H], f32r, kind="ExternalInput")
    bE = nc.dram_tensor("bE", [E, H], f32, kind="ExternalInput")
    bEr = nc.dram_tensor("bEr", [E, H], f32r, kind="ExternalInput")
    wgT = nc.dram_tensor("wgT", [I, E], f32, kind="ExternalInput")
    t3 = nc.dram_tensor("t3", [4, E], f32, kind="ExternalInput")

    out = nc.dram_tensor("out", [BCP, H], f32, kind="ExternalOutput")
    idx_out = nc.dram_tensor("idx_out", [BC, TOPK], i32, kind="ExternalOutput")

    # DRAM scratch for slot-table layout shuffles
    wdram = nc.dram_tensor("wdram", [E, CAP], f32r)
    idxdram = nc.dram_tensor("idxdram", [E, CAP], i16)

    # token b of gate tile t sits at partition p: b = 16*p + t
    xs_r = xs[0:BC, :].rearrange("(p s) i -> s p i", s=NT)
    labf_r = labf.rearrange("(p s) i -> s p i", s=NT)
    idx_out_r = idx_out.rearrange("(p s) k -> s p k", s=NT)

    with tile.TileContext(nc) as tc:
        with (
            tc.tile_pool(name="const", bufs=1) as cpool,
            tc.tile_pool(name="meta", bufs=1) as mpool,
        ):
            # ---- constants ----
            ident = cpool.tile([128, 128], f32)
            make_identity(nc, ident[:])
            iota8_i = cpool.tile([128, E], i32)
            nc.gpsimd.iota(iota8_i[:], pattern=[[1, E]], base=0, channel_multiplier=0)
            iota8 = cpool.tile([128, E], f32)
            nc.vector.tensor_copy(iota8[:], iota8_i[:])
            ones_row = cpool.tile([1, 128], f32)
            nc.vector.memset(ones_row[:], 1.0)

            wgT_sb = cpool.tile([128, IC, E], f32)
            nc.sync.dma_start(
                out=wgT_sb[:], in_=wgT.rearrange("(c p) e -> p c e", p=128)
            )

            # metadata accumulators (token b = 16p + t)
            mb_all = mpool.tile([128, E, NT], f32)   # masked token ids
            mw_all = mpool.tile([128, E, NT], f32)   # masked weights
            mbT = mpool.tile([128, 128], f32)
            mwT = mpool.tile([128, 128], f32)

            with (
                tc.tile_pool(name="gate", bufs=3) as gpool,
                tc.tile_pool(name="psA", bufs=2, space="PSUM") as psA,
                tc.tile_pool(name="psG", bufs=2, space="PSUM") as psG,
            ):
                # broadcast t3 rows across partitions via k=1 matmul
                t3b = cpool.tile([128, 3, E], f32)
                for d in range(3):
                    t3r = cpool.tile([1, E], f32, tag=f"t3r{d}")
                    nc.sync.dma_start(out=t3r[:], in_=t3[d : d + 1, :])
                    pb = psG.tile([128, E], f32, tag="pg")
                    nc.tensor.matmul(
                        out=pb[:], lhsT=ones_row[:], rhs=t3r[:],
                        start=True, stop=True,
                    )
                    nc.vector.tensor_copy(t3b[:, d, :], pb[:])

                for t in range(NT):
                    xt = gpool.tile([128, I], f32, tag="xt")
                    nc.sync.dma_start(out=xt[:], in_=xs_r[t])
                    lt = gpool.tile([128, 1], f32, tag="lt")
                    nc.sync.dma_start(out=lt[:], in_=labf_r[t])

                    # transpose x tile (8 i-chunks) for the gate matmul
                    xTg = gpool.tile([128, IC, 128], f32, tag="xTg")
                    for c in range(IC):
                        pt = psA.tile([128, 128], f32, tag="ptr")
                        nc.tensor.transpose(
                            out=pt[:], in_=xt[:, c * 128 : (c + 1) * 128],
                            identity=ident[:],
                        )
                        nc.vector.tensor_copy(xTg[:, c, :], pt[:])

                    # gate logits: true-fp32 matmul, accumulate over i-chunks
                    pg = psG.tile([128, E], f32, tag="pg")
                    for c in range(IC):
                        nc.tensor.matmul(
                            out=pg[:], lhsT=xTg[:, c, :], rhs=wgT_sb[:, c, :],
                            start=(c == 0), stop=(c == IC - 1),
                        )
                    logits = gpool.tile([128, E], f32, tag="logits")
                    nc.vector.tensor_copy(logits[:], pg[:])

                    # + difficulty table row (labels in {0,1,2})
                    for d in range(3):
                        md = gpool.tile([128, 1], f32, tag=f"md{d}")
                        nc.vector.tensor_scalar(
                            out=md[:], in0=lt[:], scalar1=float(d), scalar2=None,
                            op0=AL.is_equal,
                        )
                        nc.vector.scalar_tensor_tensor(
                            out=logits[:], in0=t3b[:, d, :], scalar=md[:, 0:1],
                            in1=logits[:], op0=AL.mult, op1=AL.add,
                        )

                    # top-2 via max8 + index match
                    mx = gpool.tile([128, E], f32, tag="mx")
                    mi = gpool.tile([128, E], u32, tag="mi")
                    nc.vector.max(mx[:], logits[:])
                    nc.vector.max_index(mi[:], mx[:], logits[:])

                    idx_pair = gpool.tile([128, TOPK], i32, tag="idxp")
                    nc.vector.tensor_copy(idx_pair[:], mi[:, 0:TOPK])
                    nc.sync.dma_start(out=idx_out_r[t], in_=idx_pair[:])

                    # softmax pieces: p1 = 1/Z, p2 = exp(v2-v1)/Z
                    nv1 = gpool.tile([128, 1], f32, tag="nv1")
                    nc.vector.tensor_scalar(
                        out=nv1[:], in0=mx[:, 0:1], scalar1=-1.0, scalar2=None,
                        op0=AL.mult,
                    )
                    exps = gpool.tile([128, E], f32, tag="exps")
                    zsum = gpool.tile([128, 1], f32, tag="zsum")
                    nc.scalar.activation(
                        out=exps[:], in_=logits[:],
                        func=mybir.ActivationFunctionType.Exp,
                        bias=nv1[:, 0:1], accum_out=zsum[:, 0:1],
                    )
                    rz = gpool.tile([128, 1], f32, tag="rz")
                    nc.vector.reciprocal(rz[:], zsum[:])
                    e2 = gpool.tile([128, 1], f32, tag="e2")
                    nc.scalar.activation(
                        out=e2[:], in_=mx[:, 1:2],
                        func=mybir.ActivationFunctionType.Exp,
                        bias=nv1[:, 0:1],
                    )
                    p2 = gpool.tile([128, 1], f32, tag="p2")
                    nc.vector.tensor_scalar(
                        out=p2[:], in0=e2[:], scalar1=rz[:, 0:1], scalar2=None,
                        op0=AL.mult,
                    )

                    # one-hots and per-expert weights
                    i1f = gpool.tile([128, 1], f32, tag="i1f")
                    i2f = gpool.tile([128, 1], f32, tag="i2f")
                    nc.vector.tensor_copy(i1f[:], mi[:, 0:1])
                    nc.vector.tensor_copy(i2f[:], mi[:, 1:2])
                    oh1 = gpool.tile([128, E], f32, tag="oh1")
                    oh2 = gpool.tile([128, E], f32, tag="oh2")
                    nc.vector.tensor_scalar(
                        out=oh1[:], in0=iota8[:], scalar1=i1f[:, 0:1], scalar2=None,
                        op0=AL.is_equal,
                    )
                    nc.vector.tensor_scalar(
                        out=oh2[:], in0=iota8[:], scalar1=i2f[:, 0:1], scalar2=None,
                        op0=AL.is_equal,
                    )
                    wt = gpool.tile([128, E], f32, tag="wt")
                    nc.vector.tensor_scalar(
                        out=wt[:], in0=oh1[:], scalar1=rz[:, 0:1], scalar2=None,
                        op0=AL.mult,
                    )
                    nc.vector.scalar_tensor_tensor(
                        out=wt[:], in0=oh2[:], scalar=p2[:, 0:1], in1=wt[:],
                        op0=AL.mult, op1=AL.add,
                    )
                    At = gpool.tile([128, E], f32, tag="At")
                    nc.vector.tensor_tensor(
                        out=At[:], in0=oh1[:], in1=oh2[:], op=AL.add
                    )

                    # masked tables: mb = A*(b+1)-1 ; mw = w + A - 1
                    b1 = gpool.tile([128, 1], i32, tag="b1")
                    nc.gpsimd.iota(
                        b1[:], pattern=[[0, 1]], base=t + 1, channel_multiplier=NT
                    )
                    b1f = gpool.tile([128, 1], f32, tag="b1f")
                    nc.vector.tensor_copy(b1f[:], b1[:])
                    nc.vector.tensor_scalar(
                        out=mb_all[:, :, t], in0=At[:], scalar1=b1f[:, 0:1],
                        scalar2=-1.0, op0=AL.mult, op1=AL.add,
                    )
                    nc.vector.scalar_tensor_tensor(
                        out=mw_all[:, :, t], in0=At[:], scalar=1.0, in1=wt[:],
                        op0=AL.subtract, op1=AL.add,
                    )

                # ---- transpose metadata to (expert, tile)-major ----
                for src, dst in ((mb_all, mbT), (mw_all, mwT)):
                    pt = psA.tile([128, 128], f32, tag="ptr")
                    nc.tensor.transpose(
                        out=pt[:], in_=src[:].rearrange("p a b -> p (a b)"),
                        identity=ident[:],
                    )
                    nc.vector.tensor_copy(dst[:], pt[:])

            # ---- compact per-expert token lists ----
            # (engine SBUF APs must start at partition 0/32/64/96, so bounce
            # each expert's 16-partition block to partition 0 via DMA first)
            # slot-position iota in the [16, F] wrapped layout: value = p + 16*f
            iosl_i = mpool.tile([16, CAP // 16], i32)
            nc.gpsimd.iota(
                iosl_i[:], pattern=[[16, CAP // 16]], base=0, channel_multiplier=1
            )
            iosl = mpool.tile([16, CAP // 16], f32)
            nc.vector.tensor_copy(iosl[:], iosl_i[:])
            ones16 = mpool.tile([1, 16], f32)
            nc.vector.memset(ones16[:], 1.0)
            zero16 = mpool.tile([16, CAP // 16], f32)
            nc.vector.memset(zero16[:], 0.0)
            pad16 = mpool.tile([16, CAP // 16], f32)
            nc.vector.memset(pad16[:], float(BC))

            cnts = mpool.tile([1, 2 * E], u32)
            with tc.tile_pool(name="psC", bufs=2, space="PSUM") as psC:
                for e in range(E):
                    sl = slice(16 * e, 16 * (e + 1))
                    mbe = mpool.tile([16, 128], f32, tag=f"mbe{e}")
                    mwe = mpool.tile([16, 128], f32, tag=f"mwe{e}")
                    nc.sync.dma_start(out=mbe[:], in_=mbT[sl, :])
                    nc.sync.dma_start(out=mwe[:], in_=mwT[sl, :])
                    idxg = mpool.tile([16, CAP // 16], f32, tag=f"idxg{e}")
                    wgg = mpool.tile([16, CAP // 16], f32, tag=f"wgg{e}")
                    nc.gpsimd.sparse_gather(
                        out=idxg[:], in_=mbe[:], num_found=cnts[:, 2 * e : 2 * e + 1]
                    )
                    nc.gpsimd.sparse_gather(
                        out=wgg[:], in_=mwe[:], num_found=cnts[:, 2 * e + 1 : 2 * e + 2]
                    )
                    # position mask: slot < count (tail of sparse_gather output
                    # is undefined on hardware, so never trust its values)
                    cntf = mpool.tile([1, 1], f32, tag=f"cntf{e}")
                    nc.vector.tensor_copy(cntf[:], cnts[:, 2 * e : 2 * e + 1])
                    pcnt = psC.tile([16, 1], f32, tag="pcnt")
                    nc.tensor.matmul(
                        out=pcnt[:], lhsT=ones16[:], rhs=cntf[:],
                        start=True, stop=True,
                    )
                    cntb = mpool.tile([16, 1], f32, tag=f"cntb{e}")
                    nc.vector.tensor_copy(cntb[:], pcnt[:])
                    mske = mpool.tile([16, CAP // 16], mybir.dt.uint8, tag=f"mske{e}")
                    nc.vector.tensor_scalar(
                        out=mske[:], in0=iosl[:], scalar1=cntb[:, 0:1],
                        scalar2=None, op0=AL.is_lt,
                    )
                    # predicated select (not multiply): the undefined tail can
                    # hold NaN patterns and NaN*0 = NaN
                    idxpe = mpool.tile([16, CAP // 16], f32, tag=f"idxpe{e}")
                    nc.vector.select(
                        out=idxpe[:], mask=mske[:], on_true=idxg[:],
                        on_false=pad16[:],
                    )
                    idx16e = mpool.tile([16, CAP // 16], i16, tag=f"idx16e{e}")
                    nc.vector.tensor_copy(idx16e[:], idxpe[:])
                    wqs = mpool.tile([16, CAP // 16], f32, tag=f"wqs{e}")
                    nc.vector.select(
                        out=wqs[:], mask=mske[:], on_true=wgg[:],
                        on_false=zero16[:],
                    )
                    wqe = mpool.tile([16, CAP // 16], f32r, tag=f"wqe{e}")
                    nc.vector.tensor_copy(wqe[:], wqs[:])
                    # bounce through DRAM to re-wrap layouts
                    nc.sync.dma_start(
                        out=wdram[e].rearrange("(f p) -> p f", p=16), in_=wqe[:]
                    )
                    nc.sync.dma_start(
                        out=idxdram[e].rearrange("(f p) -> p f", p=16), in_=idx16e[:]
                    )

            idx_rep = []  # [128, CAP//16] replicated for the 8 Q7 cores
            wcol = []     # [128, KT] weight per gathered slot (partition = slot%128)
            for e in range(E):
                rep = mpool.tile([128, CAP // 16], i16, tag=f"rep{e}")
                src = idxdram[e].rearrange("(f p) -> p f", p=16)
                for r in range(8):
                    nc.sync.dma_start(out=rep[16 * r : 16 * (r + 1), :], in_=src)
                idx_rep.append(rep)
                wc = mpool.tile([128, KT], f32, tag=f"wc{e}")
                nc.sync.dma_start(
                    out=wc[:], in_=wdram[e].rearrange("(c p) -> p c", p=128).bitcast(f32)
                )
                wcol.append(wc)

            # ---- main expert loop ----
            with (
                tc.tile_pool(name="xg", bufs=2) as xgp,
                tc.tile_pool(name="xts", bufs=2) as xtsp,
                tc.tile_pool(name="wsl", bufs=2) as wslp,
                tc.tile_pool(name="ysc", bufs=2) as yscp,
                tc.tile_pool(name="wrp", bufs=2) as wrp,
                tc.tile_pool(name="psB", bufs=2, space="PSUM") as psB,
                tc.tile_pool(name="psY", bufs=3, space="PSUM") as psY,
            ):
                for e in range(E):
                    wr = wrp.tile([1, CAP], f32r, tag="wr")
                    nc.sync.dma_start(out=wr[:], in_=wdram[e : e + 1, :])
                    xg = xgp.tile([128, KT, I], f32, tag="xg")
                    nc.gpsimd.dma_gather(
                        out_ap=xg[:], in_ap=xs[:, :], idxs_ap=idx_rep[e][:],
                        num_idxs=CAP, num_idxs_reg=CAP, elem_size=I,
                    )
                    xts = xtsp.tile([128, KT, IC, 128], f32r, tag="xts")
                    for k in range(KT):
                        nc.vector.tensor_scalar(
                            out=xg[:, k, :], in0=xg[:, k, :],
                            scalar1=wcol[e][:, k : k + 1], scalar2=None,
                            op0=AL.mult,
                        )
                        for c in range(IC):
                            pt = psB.tile([128, 128], f32, tag="ptB")
                            nc.tensor.transpose(
                                out=pt[:], in_=xg[:, k, c * 128 : (c + 1) * 128],
                                identity=ident[:],
                            )
                            nc.vector.tensor_copy(xts[:, k, c, :], pt[:])

                    for hc in range(NHC):
                        wsl = wslp.tile([128, IC, HC], f32r, tag="wsl")
                        nc.sync.dma_start(
                            out=wsl[:],
                            in_=wT[e, :, hc * HC : (hc + 1) * HC].rearrange(
                                "(c p) h -> p c h", p=128
                            ),
                        )
                        brow = wrp.tile([1, HC], f32r, tag="brow")
                        nc.sync.dma_start(
                            out=brow[:], in_=bEr[e : e + 1, hc * HC : (hc + 1) * HC]
                        )
                        ysc = yscp.tile([128, KT, HC], f32, tag="ysc")
                        for k in range(KT):
                            py = psY.tile([128, HC], f32, tag="py")
                            for c in range(IC):
                                nc.tensor.matmul(
                                    out=py[:],
                                    lhsT=xts[:, k, c, :],
                                    rhs=wsl[:, c, :],
                                    start=(c == 0), stop=False,
                                )
                            nc.tensor.matmul(
                                out=py[:],
                                lhsT=wr[:, k * 128 : (k + 1) * 128],
                                rhs=brow[:],
                                start=False, stop=True,
                            )
                            nc.vector.tensor_copy(ysc[:, k, :], py[:])
                        nc.gpsimd.dma_scatter_add(
                            out_ap=out[:, hc * HC : (hc + 1) * HC],
                            in_ap=ysc[:], idxs_ap=idx_rep[e][:],
                            num_idxs=CAP, num_idxs_reg=CAP,
                            elem_size=HC, elem_step=H,
                        )
    return nc


_NC_CACHE = {}


def _get_nc():
    if "nc" not in _NC_CACHE:
        nc = build_nc()
        nc.compile()
        _NC_CACHE["nc"] = nc
    return _NC_CACHE["nc"]


def kernel(x, W_experts, b_experts, emb_table, W_gate, b_gate, difficulty_labels):
    x = np.asarray(x, dtype=np.float32)
    W_experts = np.asarray(W_experts, dtype=np.float32)
    b_experts = np.asarray(b_experts, dtype=np.float32)
    emb_table = np.asarray(emb_table, dtype=np.float32)
    W_gate = np.asarray(W_gate, dtype=np.float32)
    b_gate = np.asarray(b_gate, dtype=np.float32)
    labels = np.asarray(difficulty_labels).astype(np.int64)

    # host-side layout prep / constant folding (weights only)
    wT = np.ascontiguousarray(W_experts.transpose(0, 2, 1))        # (E, I, H)
    wgT = np.ascontiguousarray(W_gate[:, :I].T)                    # (I, E)
    t3 = np.zeros((4, E), dtype=np.float32)
    t3[:3] = emb_table @ W_gate[:, I:].T + b_gate[None, :]         # (3, E)

    nc = _get_nc()
    in_maps = []
    for c in range(NCORES):
        sl = slice(c * BC, (c + 1) * BC)
        in_maps.append(
            {
                "xs": np.concatenate([x[sl], np.zeros((128, I), np.float32)], axis=0),
                "labf": labels[sl].astype(np.float32).reshape(BC, 1),
                "wT": wT,
                "bE": b_experts,
                "bEr": b_experts,
                "wgT": wgT,
                "t3": t3,
            }
        )

    res = run_bass_kernel_spmd(nc, in_maps, core_ids=list(range(NCORES))).results

    out = np.concatenate([res[c]["out"][:BC] for c in range(NCORES)], axis=0)
    idx = np.concatenate([res[c]["idx_out"] for c in range(NCORES)], axis=0)
    return out, idx.astype(np.int32)
